# revision 9
# baseline (speedup 1.0000x reference)
"""Trainium2 Bass kernel for multi-head attention (B=2, S=2048, D=1024, H=16).

Sharding: 8 cores = 2 (batch, data-parallel) x 4 (head-groups, tensor-parallel).
Each core (b, g) handles batch b and heads [4g, 4g+4) (a 256-wide slice of the
model dim), computing a partial output contribution; the host sums the 4
head-group partials per batch and adds the output bias.

Per-core pipeline (everything bf16 into fp32 PSUM accumulation):
  - projections: qp^T/kp^T feature-major ([d, s], W^T stationary) so the
    attention matmuls need no transposes; vp sequence-major with a fused
    bias row and a ones column per head (the ones column makes attn@V
    emit the softmax row-sums for free as PSUM row 64). The K-projection
    bias is dropped entirely: softmax is invariant to per-query constants,
    and q@bk is constant per query row.
  - attention, per (head-pair, q-block of 512): scores^T via two
    row-packed K=64 matmuls per k-tile (two heads run concurrently on
    the 128-row PE array); exp(x/8) on the scalar engine straight out of
    a 2-bank PSUM group; attn@V accumulates [65, 512] per head in PSUM.
    The k-tile loop is software-pipelined: scores(kt+1) is emitted ahead
    of attn@V(kt) so the PE never heads the queue with an exp-wait.
  - normalization: one fast DVE copy releases the accumulator, then
    reciprocal_approx_fast + gpsimd partition-broadcast + multiply.
  - output projection is cut into 8 units per q-block and interleaved
    into the next q-block's groups so its matmuls, PSUM->SBUF copies and
    output DMA all hide under the exp pipeline.
  - weights are host-prepacked into SBUF layout so every const DMA is a
    contiguous burst; DMA issue is spread across the sync/vector/gpsimd
    queues so the scalar queue stays clear for the exp pipeline.
The scalar engine paces the steady state (one 1.11us exp instruction per
k-tile); the PE array paces q-block 0 (projections are fused into its
groups) and is near-saturated elsewhere.
"""

import os
import numpy as np
import ml_dtypes

import concourse.bass as bass
import concourse.bacc as bacc
import concourse.mybir as mybir
import concourse.tile as tile
from concourse.bass_utils import run_bass_kernel_spmd

F32 = mybir.dt.float32
F32R = mybir.dt.float32r
BF16 = mybir.dt.bfloat16
AF = mybir.ActivationFunctionType

B, S, D = 2, 2048, 1024
H, DK = 16, 64
G = 4                  # head-groups (tensor parallel across cores)
DG = D // G            # 256 features per core
HPG = H // G           # 4 heads per core (2 row-packed pairs)
VEXT = HPG * (DK + 1)  # 260: per head [64 vp dims | 1 ones column]
P = 128
N_CORES = 8

_NC = None


def _build_program():
    nc = bacc.Bacc("TRN2", target_bir_lowering=False)
    qT = nc.dram_tensor("qT", [D, S], BF16, kind="ExternalInput")
    kT = nc.dram_tensor("kT", [D, S], BF16, kind="ExternalInput")
    vT = nc.dram_tensor("vT", [D, S], BF16, kind="ExternalInput")
    # host-prepacked SBUF layouts: contiguous per-partition bursts
    wqP = nc.dram_tensor("wqP", [P, 8 * DG], BF16, kind="ExternalInput")
    wkP = nc.dram_tensor("wkP", [P, 8 * DG], BF16, kind="ExternalInput")
    wvP = nc.dram_tensor("wvP", [P, 8 * VEXT], BF16, kind="ExternalInput")
    wvb = nc.dram_tensor("wvb", [1, VEXT], BF16, kind="ExternalInput")
    onesr = nc.dram_tensor("onesr", [1, P], BF16, kind="ExternalInput")
    woP = nc.dram_tensor("woP", [P, 2 * D], BF16, kind="ExternalInput")
    bqP = nc.dram_tensor("bqP", [P, 2], F32, kind="ExternalInput")
    out = nc.dram_tensor("out", [S, D], F32, kind="ExternalOutput")

    with tile.TileContext(nc) as tc:
        _body(nc, tc, qT, kT, vT, wqP, wkP, wvP, wvb, onesr, woP, bqP, out)
    nc.compile()
    return nc


def _body(nc, tc, qT, kT, vT, wqP, wkP, wvP, wvb, onesr, woP, bqP, out):
    with (
        tc.tile_pool(name="consts", bufs=1) as consts,
        tc.tile_pool(name="persist", bufs=1) as persist,
        tc.tile_pool(name="stage", bufs=6) as stage,
        tc.tile_pool(name="etp", bufs=8) as etp,
        tc.tile_pool(name="small", bufs=4) as small,
        tc.tile_pool(name="outp", bufs=8) as outp,
        tc.tile_pool(name="psA", bufs=2, space="PSUM") as psA,
        tc.tile_pool(name="psG", bufs=2, space="PSUM") as psG,
        tc.tile_pool(name="psC", bufs=1, space="PSUM") as psC,
    ):
        # --- constants / weights ---
        # critical path first, spread across queues: the first matmul needs
        # wk + the kT j0 x-block; qp needs wq + qT j0; vp needs wv + vtb.
        wk_sb = consts.tile([P, 8, DG], BF16)
        nc.sync.dma_start(wk_sb[:], wkP[:].rearrange("p (t m) -> p t m", t=8))
        wq_sb = consts.tile([P, 8, DG], BF16)
        wv_sb = consts.tile([P, 8, VEXT], BF16)
        nc.scalar.dma_start(wv_sb[:], wvP[:].rearrange("p (t m) -> p t m", t=8))
        bq_sb = consts.tile([P, 2], F32)
        wvb_sb = consts.tile([1, VEXT], BF16)
        nc.scalar.dma_start(wvb_sb[:], wvb[:])
        ones_sb = consts.tile([1, P], BF16)

        # --- persistent activations ---
        qpT_sb = persist.tile([P, 2, S], BF16)   # [d%128, d-tile(=pair), s]
        kpT_sb = persist.tile([P, 2, S], BF16)
        vp_sb = persist.tile([P, 16, VEXT], BF16)  # [s%128, s-tile, 4*(64+1)]
        an_sb = persist.tile([P, 2, S], BF16)   # normalized attn output^T

        GRP = 2  # PSUM banks per exp group (one kt, both heads)
        def ps_alloc(n, i=[0]):
            i[0] += 1
            if i[0] % 2:
                return psA.tile([P, 512], F32, tag="a", name="ps_mm")[:, :n]
            return psG.tile([P, GRP * 512], F32, tag="g", name="gps")[:, :n]

        proj_xb = {}

        def proj_dma(src_t, j, eng=None):
            xb = stage.tile([P, 8, 512], BF16, tag="xb", name="xb")
            (eng or nc.sync).dma_start(
                xb[:],
                src_t[:].rearrange("(t p) s -> p t s", p=P)[
                    :, :, j * 512 : (j + 1) * 512
                ],
            )
            return xb

        def proj_half(src_t, w_sb, b_sb, dst, j, dt):
            key = (id(src_t), j)
            if key not in proj_xb:
                proj_xb[key] = proj_dma(src_t, j)
            xb = proj_xb[key]
            ps = ps_alloc(512)
            for kt in range(8):
                nc.tensor.matmul(
                    ps[:],
                    lhsT=w_sb[:, kt, dt * P : (dt + 1) * P],
                    rhs=xb[:, kt, :],
                    start=(kt == 0),
                    stop=(kt == 7),
                )
            if b_sb is None:
                nc.vector.tensor_copy(dst[:, dt, j * 512 : (j + 1) * 512], ps[:])
            else:
                nc.vector.tensor_scalar_add(
                    dst[:, dt, j * 512 : (j + 1) * 512], ps[:], b_sb[:, dt : dt + 1]
                )

        vtb_cache = {}

        def vp_block(st):
            # two s-tiles per DMA: 1KB bursts instead of 512B, half the loads
            st0 = st - st % 2
            if st0 not in vtb_cache:
                vtb2 = stage.tile([P, 8, 2 * P], BF16, tag="vtb", name="vtb")
                nc.sync.dma_start(
                    vtb2[:],
                    vT[:].rearrange("(t p) s -> p t s", p=P)[
                        :, :, st0 * P : (st0 + 2) * P
                    ],
                )
                vtb_cache[st0] = vtb2
            vtb = vtb_cache[st0]
            off = (st - st0) * P
            psv = ps_alloc(VEXT)
            for kt in range(8):
                nc.tensor.matmul(
                    psv[:],
                    lhsT=vtb[:, kt, off : off + P],
                    rhs=wv_sb[:, kt, :],
                    start=(kt == 0),
                    stop=(kt == 7),
                )
            nc.vector.tensor_tensor(
                vp_sb[:, st, :], psv[:], wvb_bc[:], mybir.AluOpType.add
            )

        wo_sb = consts.tile([P, 2, D], BF16)
        nc.scalar.dma_start(wo_sb[:], woP[:].rearrange("p (t o) -> p t o", t=2))

        # prefetch the kT/qT j0 x-blocks + q-side weights on the gpsimd
        # queue so they overlap the sync/scalar-queue weight loads
        proj_xb[(id(kT), 0)] = proj_dma(kT, 0, eng=nc.gpsimd)
        nc.gpsimd.dma_start(wq_sb[:], wqP[:].rearrange("p (t m) -> p t m", t=8))
        proj_xb[(id(qT), 0)] = proj_dma(qT, 0, eng=nc.gpsimd)
        nc.gpsimd.dma_start(bq_sb[:], bqP[:])
        nc.gpsimd.dma_start(ones_sb[:], onesr[:])

        # bias+ones row broadcast once to all partitions (folded into the
        # vp PSUM->SBUF copy as a vector add, replacing 16 K=1 matmuls)
        wvb_bc = consts.tile([P, VEXT], BF16)
        nc.gpsimd.partition_broadcast(wvb_bc[:], wvb_sb[:])

        # warm the ACT exp table early so the ~2.7us load overlaps phase 1
        warm = consts.tile([1, 8], F32)
        nc.vector.memset(warm[:], 0.0)
        nc.scalar.activation(warm[:], warm[:], AF.Exp)

        # bootstrap: just enough of kp/qp/vp for (qb0, pair0, kt=0,1);
        # the rest of the projections are emitted inside qb0/pair0 below,
        # hiding their PE time under the exp pipeline instead of idling ACT
        proj_half(kT, wk_sb, None, kpT_sb, 0, 0)
        proj_half(qT, wq_sb, bq_sb, qpT_sb, 0, 0)
        vp_block(0)
        vp_block(1)

        # insertion schedule for qb0/pair0: at group kt, emit these blocks
        fuse0 = {kt: [] for kt in range(16)}
        for kt in range(14):
            fuse0[kt].append(("vp", kt + 2))
        for j in (1, 2, 3):
            fuse0[4 * j - 2].append(("kp", j, 0))   # needed at group 4j
        fuse0[2].append(("kp", 0, 1))               # pair1 kp j0
        fuse0[12].append(("qp", 0, 1))              # qp j0 dt1 for pair1
        # pair1's kp j1-3 stream just-in-time inside qb0/pair1 instead
        fuse1 = {kt: [] for kt in range(16)}
        for j in (1, 2, 3):
            fuse1[4 * j - 2].append(("kp", j, 1))

        # --- phase 2 per q-block; qp^T j-block emitted just-in-time ---
        def d_unit(qb, u, ring=0):
            # one (q-tile, out-half) unit of the output projection for block qb
            qt, o = u // 2, u % 2
            q0 = qb * 512 + qt * P
            dps = psA.tile([P, 512], F32, tag="a", name="dps")
            for p2 in range(2):
                nc.tensor.matmul(
                    dps[:],
                    lhsT=an_sb[:, p2, q0 : q0 + P],
                    rhs=wo_sb[:, p2, o * 512 : (o + 1) * 512],
                    start=(p2 == 0),
                    stop=(p2 == 1),
                )
            osb = outp.tile([P, 512], F32, tag="o")
            nc.vector.tensor_copy(osb[:], dps[:])
            eng = (nc.sync, nc.gpsimd)[ring % 2]
            eng.dma_start(out[q0 : q0 + P, o * 512 : (o + 1) * 512], osb[:])

        # flat software-pipelined schedule over all (qb, pair, kt) steps:
        # scores(step+1) is emitted ahead of exp/attnV(step).
        steps = [(qb, pair, kt) for qb in range(4) for pair in range(2)
                 for kt in range(16)]
        cc_tiles = {}

        def get_cc(qb, pair):
            key = (qb, pair)
            if key not in cc_tiles:
                cc_tiles[key] = psC.tile([DK + 1, 1024], F32, tag="c", name="cc")
            return cc_tiles[key]

        def emit_scores(qb, pair, kt):
            qs = slice(qb * 512, (qb + 1) * 512)
            gps = psG.tile([P, GRP * 512], F32, tag="g", name="gps")
            for hh in range(2):
                hp = slice(hh * DK, (hh + 1) * DK)
                nc.tensor.matmul(
                    gps[:, hh * 512 : (hh + 1) * 512],
                    lhsT=kpT_sb[hp, pair, kt * P : (kt + 1) * P],
                    rhs=qpT_sb[hp, pair, qs],
                    start=True,
                    stop=True,
                )
            return gps

        def c_mms(qb, pair, kt, et):
            cc = get_cc(qb, pair)
            for hh in range(2):
                h = 2 * pair + hh
                nc.tensor.matmul(
                    cc[:, hh * 512 : (hh + 1) * 512],
                    lhsT=vp_sb[:, kt, h * (DK + 1) : (h + 1) * (DK + 1)],
                    rhs=et[:, hh * 512 : (hh + 1) * 512],
                    start=(kt == 0),
                    stop=(kt == 15),
                )

        def normalize(qb, pair):
            # single fast copy releases the PSUM accumulator; normalize
            # (reciprocal of row 64, broadcast, multiply) runs from SBUF
            qs = slice(qb * 512, (qb + 1) * 512)
            cc = cc_tiles.pop((qb, pair))
            csb = small.tile([DK + 1, 1024], F32, tag="csb")
            nc.vector.tensor_copy(csb[:], cc[:])
            rbcs = []
            for hh in range(2):
                cs = slice(hh * 512, (hh + 1) * 512)
                rsum = small.tile([1, 512], F32, tag="rsum")
                nc.vector.tensor_copy(rsum[:], csb[DK : DK + 1, cs])
                rinv = small.tile([1, 512], F32, tag="rinv")
                nc.vector.reciprocal_approx_fast(rinv[:], rsum[:])
                rbc = small.tile([DK, 512], F32, tag="rbc")
                nc.gpsimd.partition_broadcast(rbc[:], rinv[:])
                rbcs.append(rbc)
            for hh in range(2):
                cs = slice(hh * 512, (hh + 1) * 512)
                nc.vector.tensor_tensor(
                    an_sb[hh * DK : (hh + 1) * DK, pair, qs],
                    csb[:DK, cs],
                    rbcs[hh],
                    mybir.AluOpType.mult,
                )

        def normalize_final(qb, pair):
            # last pair: no successor waits on the PSUM banks, so read PSUM
            # directly; interleave the two head chains and cut the multiply
            # into q-halves so the output projection starts ASAP.
            qs0 = qb * 512
            cc = cc_tiles.pop((qb, pair))
            rinvs = []
            for hh in range(2):
                rsum = small.tile([1, 512], F32, tag="rsum")
                nc.vector.tensor_copy(rsum[:], cc[DK : DK + 1, hh * 512 : (hh + 1) * 512])
                rinv = small.tile([1, 512], F32, tag="rinv")
                nc.vector.reciprocal_approx_fast(rinv[:], rsum[:])
                rinvs.append(rinv)
            rbcs = []
            for hh in range(2):
                rbc = small.tile([DK, 512], F32, tag="rbc")
                nc.gpsimd.partition_broadcast(rbc[:], rinvs[hh])
                rbcs.append(rbc)
            for half in range(2):
                h0 = half * 256
                for hh in range(2):
                    nc.vector.tensor_tensor(
                        an_sb[hh * DK : (hh + 1) * DK, pair,
                              qs0 + h0 : qs0 + h0 + 256],
                        cc[:DK, hh * 512 + h0 : hh * 512 + h0 + 256],
                        rbcs[hh][:, h0 : h0 + 256],
                        mybir.AluOpType.mult,
                    )
                for qt in (2 * half, 2 * half + 1):
                    d_unit(qb, 2 * qt, ring=qt)
                    d_unit(qb, 2 * qt + 1, ring=qt + 1)

        # spread the previous q-block's output projection through each
        # pair's groups so the scalar engine never starves
        d_sched = {3: 0, 7: 1, 11: 2, 14: 3}

        gps_next = emit_scores(0, 0, 0)
        for si, (qb, pair, kt) in enumerate(steps):
            gps = gps_next
            if si + 1 < len(steps):
                gps_next = emit_scores(*steps[si + 1])
            et = etp.tile([P, GRP * 512], BF16, tag="e", name="et")
            nc.scalar.activation(et[:], gps[:], AF.Exp, scale=1.0 / np.sqrt(DK))
            c_mms(qb, pair, kt, et)
            if qb > 0 and kt in d_sched:
                u = 4 * pair + d_sched[kt]
                d_unit(qb - 1, u, ring=u)
            if qb == 0 and pair == 0:
                for item in fuse0[kt]:
                    if item[0] == "vp":
                        vp_block(item[1])
                    elif item[0] == "kp":
                        proj_half(kT, wk_sb, None, kpT_sb, item[1], item[2])
                    else:
                        proj_half(qT, wq_sb, bq_sb, qpT_sb, item[1], item[2])
            if qb == 0 and pair == 1:
                for item in fuse1[kt]:
                    proj_half(kT, wk_sb, None, kpT_sb, item[1], item[2])
            if pair == 1 and qb < 3 and kt in (1, 9):
                proj_half(qT, wq_sb, bq_sb, qpT_sb, qb + 1, kt // 8)
            if kt == 15:
                if (qb, pair) == (3, 1):
                    normalize_final(qb, pair)
                else:
                    normalize(qb, pair)


def _get_program():
    global _NC
    if _NC is None:
        _NC = _build_program()
    return _NC


def _make_in_maps(v, k, q, Wv, bv, Wk, bk, Wq, bq, Wo, bo):
    f32 = np.float32
    bf16 = ml_dtypes.bfloat16
    qT = [np.ascontiguousarray(q[b].T).astype(bf16) for b in range(B)]
    kT = [np.ascontiguousarray(k[b].T).astype(bf16) for b in range(B)]
    vT = [np.ascontiguousarray(v[b].T).astype(bf16) for b in range(B)]

    def prepack(w2d):
        # [D, M] -> [P, 8*M]: row t*128+p lands at partition p, block t
        Dd, M = w2d.shape
        return np.ascontiguousarray(
            w2d.reshape(8, P, M).transpose(1, 0, 2).reshape(P, 8 * M)
        )

    per_group = []
    for g in range(G):
        gs = slice(g * DG, (g + 1) * DG)
        wqT = prepack(Wq[gs, :].T.astype(f32)).astype(bf16)
        wkT = prepack(Wk[gs, :].T.astype(f32)).astype(bf16)
        wvm = np.zeros((D, VEXT), dtype=f32)
        wvb = np.zeros((1, VEXT), dtype=f32)
        for h in range(HPG):
            cs = slice(h * (DK + 1), h * (DK + 1) + DK)
            rows = slice(g * DG + h * DK, g * DG + (h + 1) * DK)
            wvm[:, cs] = Wv[rows, :].T
            wvb[0, cs] = bv[rows]
            wvb[0, h * (DK + 1) + DK] = 1.0
        wvP = prepack(wvm).astype(bf16)
        wvb = wvb.astype(bf16)
        # [DG, D] -> [P, 2*D]
        woT = np.ascontiguousarray(Wo[:, gs].T)
        woP = np.ascontiguousarray(
            woT.reshape(2, P, D).transpose(1, 0, 2).reshape(P, 2 * D)
        ).astype(bf16)
        bqP = np.ascontiguousarray(
            bq[gs].reshape(2, P).T, dtype=f32
        )
        per_group.append(
            dict(wqP=wqT, wkP=wkT, wvP=wvP, wvb=wvb, woP=woP, bqP=bqP)
        )

    in_maps = []
    for c in range(N_CORES):
        b, g = c // G, c % G
        m = dict(qT=qT[b], kT=kT[b], vT=vT[b],
                 onesr=np.ones((1, P), dtype=bf16), **per_group[g])
        in_maps.append(m)
    return in_maps


def _gather(results, bo):
    out = np.zeros((B, S, D), dtype=np.float32)
    for c in range(N_CORES):
        b = c // G
        out[b] += results[c]["out"]
    out += bo.astype(np.float32)
    return out


def run(v, k, q, Wv, bv, Wk, bk, Wq, bq, Wo, bo, trace=False):
    nc = _get_program()
    in_maps = _make_in_maps(v, k, q, Wv, bv, Wk, bk, Wq, bq, Wo, bo)
    res = run_bass_kernel_spmd(
        nc, in_maps, core_ids=list(range(N_CORES)), trace=trace
    )
    return _gather(res.results, np.asarray(bo)), res


def kernel(v, k, q, Wv, bv, Wk, bk, Wq, bq, Wo, bo):
    args = [np.asarray(x, dtype=np.float32)
            for x in (v, k, q, Wv, bv, Wk, bk, Wq, bq, Wo, bo)]
    out, _ = run(*args, trace=bool(int(os.environ.get("MHA_TRACE", "0"))))
    return out


# revision 13
# speedup vs baseline: 1.0232x; 1.0232x over previous
"""Trainium2 Bass kernel for multi-head attention (B=2, S=2048, D=1024, H=16).

Sharding: 8 cores = 2 (batch, data-parallel) x 4 (head-groups, tensor-parallel).
Each core (b, g) handles batch b and heads [4g, 4g+4) (a 256-wide slice of the
model dim), computing a partial output contribution; the host sums the 4
head-group partials per batch and adds the output bias.

Per-core pipeline (everything bf16 into fp32 PSUM accumulation):
  - projections: qp^T/kp^T feature-major ([d, s], W^T stationary) so the
    attention matmuls need no transposes; vp sequence-major with a fused
    bias row and a ones column per head (the ones column makes attn@V
    emit the softmax row-sums for free as PSUM row 64). The K-projection
    bias is dropped entirely: softmax is invariant to per-query constants,
    and q@bk is constant per query row.
  - attention, per (head-pair, q-block of 512): scores^T via two
    row-packed K=64 matmuls per k-tile (two heads run concurrently on
    the 128-row PE array); exp(x/8) on the scalar engine straight out of
    a 2-bank PSUM group; attn@V accumulates [65, 512] per head in PSUM.
    The k-tile loop is software-pipelined: scores(kt+1) is emitted ahead
    of attn@V(kt) so the PE never heads the queue with an exp-wait.
  - normalization: one fast DVE copy releases the accumulator, then
    reciprocal_approx_fast + gpsimd partition-broadcast + multiply.
  - output projection is cut into 8 units per q-block and interleaved
    into the next q-block's groups so its matmuls, PSUM->SBUF copies and
    output DMA all hide under the exp pipeline.
  - weights are host-prepacked into SBUF layout so every const DMA is a
    contiguous burst; DMA issue is spread across the sync/vector/gpsimd
    queues so the scalar queue stays clear for the exp pipeline.
The scalar engine paces the steady state (one 1.11us exp instruction per
k-tile); the PE array paces q-block 0 (projections are fused into its
groups) and is near-saturated elsewhere.
"""

import os
import numpy as np
import ml_dtypes

import concourse.bass as bass
import concourse.bacc as bacc
import concourse.mybir as mybir
import concourse.tile as tile
from concourse.bass_utils import run_bass_kernel_spmd

F32 = mybir.dt.float32
F32R = mybir.dt.float32r
BF16 = mybir.dt.bfloat16
AF = mybir.ActivationFunctionType

B, S, D = 2, 2048, 1024
H, DK = 16, 64
G = 4                  # head-groups (tensor parallel across cores)
DG = D // G            # 256 features per core
HPG = H // G           # 4 heads per core (2 row-packed pairs)
VEXT = HPG * (DK + 1)  # 260: per head [64 vp dims | 1 ones column]
P = 128
N_CORES = 8

_NC = None


def _build_program():
    nc = bacc.Bacc("TRN2", target_bir_lowering=False)
    qT = nc.dram_tensor("qT", [D, S], BF16, kind="ExternalInput")
    kT = nc.dram_tensor("kT", [D, S], BF16, kind="ExternalInput")
    vT = nc.dram_tensor("vT", [D, S], BF16, kind="ExternalInput")
    # host-prepacked SBUF layouts: contiguous per-partition bursts
    wqP = nc.dram_tensor("wqP", [P, 8 * DG], BF16, kind="ExternalInput")
    wkP = nc.dram_tensor("wkP", [P, 8 * DG], BF16, kind="ExternalInput")
    wvP = nc.dram_tensor("wvP", [P, 8 * VEXT], BF16, kind="ExternalInput")
    wvb = nc.dram_tensor("wvb", [1, VEXT], BF16, kind="ExternalInput")
    onesr = nc.dram_tensor("onesr", [1, P], BF16, kind="ExternalInput")
    woP = nc.dram_tensor("woP", [P, 2 * D], BF16, kind="ExternalInput")
    bqP = nc.dram_tensor("bqP", [P, 2], F32, kind="ExternalInput")
    out = nc.dram_tensor("out", [S, D], F32, kind="ExternalOutput")

    with tile.TileContext(nc) as tc:
        _body(nc, tc, qT, kT, vT, wqP, wkP, wvP, wvb, onesr, woP, bqP, out)
    nc.compile()
    return nc


def _body(nc, tc, qT, kT, vT, wqP, wkP, wvP, wvb, onesr, woP, bqP, out):
    with (
        tc.tile_pool(name="consts", bufs=1) as consts,
        tc.tile_pool(name="persist", bufs=1) as persist,
        tc.tile_pool(name="stage", bufs=6) as stage,
        tc.tile_pool(name="etp", bufs=8) as etp,
        tc.tile_pool(name="small", bufs=4) as small,
        tc.tile_pool(name="outp", bufs=8) as outp,
        tc.tile_pool(name="psA", bufs=2, space="PSUM") as psA,
        tc.tile_pool(name="psG", bufs=2, space="PSUM") as psG,
        tc.tile_pool(name="psC", bufs=1, space="PSUM") as psC,
    ):
        # --- constants / weights ---
        # critical path first, spread across queues: the first matmul needs
        # wk + the kT j0 x-block; qp needs wq + qT j0; vp needs wv + vtb.
        wk_sb = consts.tile([P, 8, DG], BF16)
        nc.scalar.dma_start(wk_sb[:], wkP[:].rearrange("p (t m) -> p t m", t=8))
        wq_sb = consts.tile([P, 8, DG], BF16)
        nc.scalar.dma_start(wq_sb[:], wqP[:].rearrange("p (t m) -> p t m", t=8))
        wv_sb = consts.tile([P, 8, VEXT], BF16)
        nc.scalar.dma_start(wv_sb[:], wvP[:].rearrange("p (t m) -> p t m", t=8))
        bq_sb = consts.tile([P, 2], F32)
        nc.scalar.dma_start(bq_sb[:], bqP[:])
        wvb_sb = consts.tile([1, VEXT], BF16)
        nc.scalar.dma_start(wvb_sb[:], wvb[:])
        ones_sb = consts.tile([1, P], BF16)
        nc.scalar.dma_start(ones_sb[:], onesr[:])

        # --- persistent activations ---
        qpT_sb = persist.tile([P, 2, S], BF16)   # [d%128, d-tile(=pair), s]
        kpT_sb = persist.tile([P, 2, S], BF16)
        vp_sb = persist.tile([P, 16, VEXT], BF16)  # [s%128, s-tile, 4*(64+1)]
        an_sb = persist.tile([P, 2, S], BF16)   # normalized attn output^T

        GRP = 2  # PSUM banks per exp group (one kt, both heads)
        def ps_alloc(n, i=[0]):
            i[0] += 1
            if i[0] % 2:
                return psA.tile([P, 512], F32, tag="a", name="ps_mm")[:, :n]
            return psG.tile([P, GRP * 512], F32, tag="g", name="gps")[:, :n]

        proj_xb = {}

        def proj_dma(src_t, j, eng=None):
            xb = stage.tile([P, 8, 512], BF16, tag="xb", name="xb")
            (eng or nc.sync).dma_start(
                xb[:],
                src_t[:].rearrange("(t p) s -> p t s", p=P)[
                    :, :, j * 512 : (j + 1) * 512
                ],
            )
            return xb

        def proj_half(src_t, w_sb, b_sb, dst, j, dt):
            key = (id(src_t), j)
            if key not in proj_xb:
                proj_xb[key] = proj_dma(src_t, j)
            xb = proj_xb[key]
            ps = ps_alloc(512)
            for kt in range(8):
                nc.tensor.matmul(
                    ps[:],
                    lhsT=w_sb[:, kt, dt * P : (dt + 1) * P],
                    rhs=xb[:, kt, :],
                    start=(kt == 0),
                    stop=(kt == 7),
                )
            if b_sb is None:
                nc.vector.tensor_copy(dst[:, dt, j * 512 : (j + 1) * 512], ps[:])
            else:
                nc.vector.tensor_scalar_add(
                    dst[:, dt, j * 512 : (j + 1) * 512], ps[:], b_sb[:, dt : dt + 1]
                )

        vtb_cache = {}

        def vp_block(st):
            # two s-tiles per DMA: 1KB bursts instead of 512B, half the loads
            st0 = st - st % 2
            if st0 not in vtb_cache:
                vtb2 = stage.tile([P, 8, 2 * P], BF16, tag="vtb", name="vtb")
                nc.sync.dma_start(
                    vtb2[:],
                    vT[:].rearrange("(t p) s -> p t s", p=P)[
                        :, :, st0 * P : (st0 + 2) * P
                    ],
                )
                vtb_cache[st0] = vtb2
            vtb = vtb_cache[st0]
            off = (st - st0) * P
            psv = ps_alloc(VEXT)
            for kt in range(8):
                nc.tensor.matmul(
                    psv[:],
                    lhsT=vtb[:, kt, off : off + P],
                    rhs=wv_sb[:, kt, :],
                    start=(kt == 0),
                    stop=(kt == 7),
                )
            nc.vector.tensor_tensor(
                vp_sb[:, st, :], psv[:], wvb_bc[:], mybir.AluOpType.add
            )

        wo_sb = consts.tile([P, 2, D], BF16)
        nc.scalar.dma_start(wo_sb[:], woP[:].rearrange("p (t o) -> p t o", t=2))

        # prefetch the kT/qT j0 x-blocks on the sync queue right away
        proj_xb[(id(kT), 0)] = proj_dma(kT, 0, eng=nc.sync)
        proj_xb[(id(qT), 0)] = proj_dma(qT, 0, eng=nc.sync)

        # bias+ones row broadcast once to all partitions (folded into the
        # vp PSUM->SBUF copy as a vector add, replacing 16 K=1 matmuls)
        wvb_bc = consts.tile([P, VEXT], BF16)
        nc.gpsimd.partition_broadcast(wvb_bc[:], wvb_sb[:])

        # warm the ACT exp table early so the ~2.7us load overlaps phase 1
        warm = consts.tile([1, 8], F32)
        nc.vector.memset(warm[:], 0.0)
        nc.scalar.activation(warm[:], warm[:], AF.Exp)

        # bootstrap: just enough of kp/qp/vp for (qb0, pair0, kt=0,1);
        # the rest of the projections are emitted inside qb0/pair0 below,
        # hiding their PE time under the exp pipeline instead of idling ACT
        proj_half(kT, wk_sb, None, kpT_sb, 0, 0)
        proj_half(qT, wq_sb, bq_sb, qpT_sb, 0, 0)
        vp_block(0)
        vp_block(1)

        # insertion schedule for qb0/pair0: at group kt, emit these blocks
        fuse0 = {kt: [] for kt in range(16)}
        for kt in range(14):
            fuse0[kt].append(("vp", kt + 2))
        for j in (1, 2, 3):
            fuse0[4 * j - 2].append(("kp", j, 0))   # needed at group 4j
        fuse0[2].append(("kp", 0, 1))               # pair1 kp j0
        fuse0[12].append(("qp", 0, 1))              # qp j0 dt1 for pair1
        # pair1's kp j1-3 stream just-in-time inside qb0/pair1 instead
        fuse1 = {kt: [] for kt in range(16)}
        for j in (1, 2, 3):
            fuse1[4 * j - 2].append(("kp", j, 1))

        # --- phase 2 per q-block; qp^T j-block emitted just-in-time ---
        def d_unit(qb, u, ring=0, final=False):
            # one (q-tile, out-half) unit of the output projection for block qb
            qt, o = u // 2, u % 2
            q0 = qb * 512 + qt * P
            dps = psA.tile([P, 512], F32, tag="a", name="dps")
            for p2 in range(2):
                nc.tensor.matmul(
                    dps[:],
                    lhsT=an_sb[:, p2, q0 : q0 + P],
                    rhs=wo_sb[:, p2, o * 512 : (o + 1) * 512],
                    start=(p2 == 0),
                    stop=(p2 == 1),
                )
            osb = outp.tile([P, 512], F32, tag="o")
            nc.vector.tensor_copy(osb[:], dps[:])
            eng = (nc.sync, nc.scalar if final else nc.gpsimd)[ring % 2]
            eng.dma_start(out[q0 : q0 + P, o * 512 : (o + 1) * 512], osb[:])

        # flat software-pipelined schedule over all (qb, pair, kt) steps:
        # scores(step+1) is emitted ahead of exp/attnV(step).
        steps = [(qb, pair, kt) for qb in range(4) for pair in range(2)
                 for kt in range(16)]
        cc_tiles = {}

        def get_cc(qb, pair):
            key = (qb, pair)
            if key not in cc_tiles:
                cc_tiles[key] = psC.tile([DK + 1, 1024], F32, tag="c", name="cc")
            return cc_tiles[key]

        def emit_scores(qb, pair, kt):
            qs = slice(qb * 512, (qb + 1) * 512)
            gps = psG.tile([P, GRP * 512], F32, tag="g", name="gps")
            for hh in range(2):
                hp = slice(hh * DK, (hh + 1) * DK)
                nc.tensor.matmul(
                    gps[:, hh * 512 : (hh + 1) * 512],
                    lhsT=kpT_sb[hp, pair, kt * P : (kt + 1) * P],
                    rhs=qpT_sb[hp, pair, qs],
                    start=True,
                    stop=True,
                )
            return gps

        def c_mms(qb, pair, kt, et):
            cc = get_cc(qb, pair)
            for hh in range(2):
                h = 2 * pair + hh
                nc.tensor.matmul(
                    cc[:, hh * 512 : (hh + 1) * 512],
                    lhsT=vp_sb[:, kt, h * (DK + 1) : (h + 1) * (DK + 1)],
                    rhs=et[:, hh * 512 : (hh + 1) * 512],
                    start=(kt == 0),
                    stop=(kt == 15),
                )

        def normalize(qb, pair):
            # single fast copy releases the PSUM accumulator; normalize
            # (reciprocal of row 64, broadcast, multiply) runs from SBUF
            qs = slice(qb * 512, (qb + 1) * 512)
            cc = cc_tiles.pop((qb, pair))
            csb = small.tile([DK + 1, 1024], F32, tag="csb")
            nc.vector.tensor_copy(csb[:], cc[:])
            rbcs = []
            for hh in range(2):
                cs = slice(hh * 512, (hh + 1) * 512)
                rsum = small.tile([1, 512], F32, tag="rsum")
                nc.vector.tensor_copy(rsum[:], csb[DK : DK + 1, cs])
                rinv = small.tile([1, 512], F32, tag="rinv")
                nc.vector.reciprocal_approx_fast(rinv[:], rsum[:])
                rbc = small.tile([DK, 512], F32, tag="rbc")
                nc.gpsimd.partition_broadcast(rbc[:], rinv[:])
                rbcs.append(rbc)
            for hh in range(2):
                cs = slice(hh * 512, (hh + 1) * 512)
                nc.vector.tensor_tensor(
                    an_sb[hh * DK : (hh + 1) * DK, pair, qs],
                    csb[:DK, cs],
                    rbcs[hh],
                    mybir.AluOpType.mult,
                )

        def normalize_final(qb, pair):
            # last pair: no successor waits on the PSUM banks, so read PSUM
            # directly; interleave the two head chains and cut the multiply
            # into q-halves so the output projection starts ASAP.
            qs0 = qb * 512
            cc = cc_tiles.pop((qb, pair))
            rinvs = []
            for hh in range(2):
                rsum = small.tile([1, 512], F32, tag="rsum")
                nc.vector.tensor_copy(rsum[:], cc[DK : DK + 1, hh * 512 : (hh + 1) * 512])
                rinv = small.tile([1, 512], F32, tag="rinv")
                nc.vector.reciprocal_approx_fast(rinv[:], rsum[:])
                rinvs.append(rinv)
            rbcs = []
            for hh in range(2):
                rbc = small.tile([DK, 512], F32, tag="rbc")
                nc.gpsimd.partition_broadcast(rbc[:], rinvs[hh])
                rbcs.append(rbc)
            for half in range(2):
                h0 = half * 256
                for hh in range(2):
                    nc.vector.tensor_tensor(
                        an_sb[hh * DK : (hh + 1) * DK, pair,
                              qs0 + h0 : qs0 + h0 + 256],
                        cc[:DK, hh * 512 + h0 : hh * 512 + h0 + 256],
                        rbcs[hh][:, h0 : h0 + 256],
                        mybir.AluOpType.mult,
                    )
                for qt in (2 * half, 2 * half + 1):
                    d_unit(qb, 2 * qt, ring=qt, final=True)
                    d_unit(qb, 2 * qt + 1, ring=qt + 1, final=True)

        # spread the previous q-block's output projection through each
        # pair's groups so the scalar engine never starves
        d_sched = {3: 0, 7: 1, 11: 2, 14: 3}

        gps_next = emit_scores(0, 0, 0)
        for si, (qb, pair, kt) in enumerate(steps):
            gps = gps_next
            if si + 1 < len(steps):
                gps_next = emit_scores(*steps[si + 1])
            et = etp.tile([P, GRP * 512], BF16, tag="e", name="et")
            nc.scalar.activation(et[:], gps[:], AF.Exp, scale=1.0 / np.sqrt(DK))
            c_mms(qb, pair, kt, et)
            if qb > 0 and kt in d_sched:
                u = 4 * pair + d_sched[kt]
                d_unit(qb - 1, u, ring=u)
            if qb == 0 and pair == 0:
                for item in fuse0[kt]:
                    if item[0] == "vp":
                        vp_block(item[1])
                    elif item[0] == "kp":
                        proj_half(kT, wk_sb, None, kpT_sb, item[1], item[2])
                    else:
                        proj_half(qT, wq_sb, bq_sb, qpT_sb, item[1], item[2])
            if qb == 0 and pair == 1:
                for item in fuse1[kt]:
                    proj_half(kT, wk_sb, None, kpT_sb, item[1], item[2])
            if pair == 1 and qb < 3 and kt in (1, 9):
                proj_half(qT, wq_sb, bq_sb, qpT_sb, qb + 1, kt // 8)
            if kt == 15:
                if (qb, pair) == (3, 1):
                    normalize_final(qb, pair)
                else:
                    normalize(qb, pair)


def _get_program():
    global _NC
    if _NC is None:
        _NC = _build_program()
    return _NC


def _make_in_maps(v, k, q, Wv, bv, Wk, bk, Wq, bq, Wo, bo):
    f32 = np.float32
    bf16 = ml_dtypes.bfloat16
    qT = [np.ascontiguousarray(q[b].T).astype(bf16) for b in range(B)]
    kT = [np.ascontiguousarray(k[b].T).astype(bf16) for b in range(B)]
    vT = [np.ascontiguousarray(v[b].T).astype(bf16) for b in range(B)]

    def prepack(w2d):
        # [D, M] -> [P, 8*M]: row t*128+p lands at partition p, block t
        Dd, M = w2d.shape
        return np.ascontiguousarray(
            w2d.reshape(8, P, M).transpose(1, 0, 2).reshape(P, 8 * M)
        )

    per_group = []
    for g in range(G):
        gs = slice(g * DG, (g + 1) * DG)
        wqT = prepack(Wq[gs, :].T.astype(f32)).astype(bf16)
        wkT = prepack(Wk[gs, :].T.astype(f32)).astype(bf16)
        wvm = np.zeros((D, VEXT), dtype=f32)
        wvb = np.zeros((1, VEXT), dtype=f32)
        for h in range(HPG):
            cs = slice(h * (DK + 1), h * (DK + 1) + DK)
            rows = slice(g * DG + h * DK, g * DG + (h + 1) * DK)
            wvm[:, cs] = Wv[rows, :].T
            wvb[0, cs] = bv[rows]
            wvb[0, h * (DK + 1) + DK] = 1.0
        wvP = prepack(wvm).astype(bf16)
        wvb = wvb.astype(bf16)
        # [DG, D] -> [P, 2*D]
        woT = np.ascontiguousarray(Wo[:, gs].T)
        woP = np.ascontiguousarray(
            woT.reshape(2, P, D).transpose(1, 0, 2).reshape(P, 2 * D)
        ).astype(bf16)
        bqP = np.ascontiguousarray(
            bq[gs].reshape(2, P).T, dtype=f32
        )
        per_group.append(
            dict(wqP=wqT, wkP=wkT, wvP=wvP, wvb=wvb, woP=woP, bqP=bqP)
        )

    in_maps = []
    for c in range(N_CORES):
        b, g = c // G, c % G
        m = dict(qT=qT[b], kT=kT[b], vT=vT[b],
                 onesr=np.ones((1, P), dtype=bf16), **per_group[g])
        in_maps.append(m)
    return in_maps


def _gather(results, bo):
    out = np.zeros((B, S, D), dtype=np.float32)
    for c in range(N_CORES):
        b = c // G
        out[b] += results[c]["out"]
    out += bo.astype(np.float32)
    return out


def run(v, k, q, Wv, bv, Wk, bk, Wq, bq, Wo, bo, trace=False):
    nc = _get_program()
    in_maps = _make_in_maps(v, k, q, Wv, bv, Wk, bk, Wq, bq, Wo, bo)
    res = run_bass_kernel_spmd(
        nc, in_maps, core_ids=list(range(N_CORES)), trace=trace
    )
    return _gather(res.results, np.asarray(bo)), res


def kernel(v, k, q, Wv, bv, Wk, bk, Wq, bq, Wo, bo):
    args = [np.asarray(x, dtype=np.float32)
            for x in (v, k, q, Wv, bv, Wk, bk, Wq, bq, Wo, bo)]
    out, _ = run(*args, trace=bool(int(os.environ.get("MHA_TRACE", "0"))))
    return out


# revision 25
# speedup vs baseline: 1.0633x; 1.0391x over previous
"""Trainium2 Bass kernel for multi-head attention (B=2, S=2048, D=1024, H=16).

Sharding: 8 cores = 2 (batch, data-parallel) x 4 (head-groups, tensor-parallel).
Each core (b, g) handles batch b and heads [4g, 4g+4) (a 256-wide slice of the
model dim), computing a partial output contribution; the host sums the 4
head-group partials per batch and adds the output bias.

Per-core pipeline (everything bf16 into fp32 PSUM accumulation):
  - projections: qp^T/kp^T feature-major ([d, s], W^T stationary) so the
    attention matmuls need no transposes; vp sequence-major with a fused
    bias row and a ones column per head (the ones column makes attn@V
    emit the softmax row-sums for free as PSUM row 64). The K-projection
    bias is dropped entirely: softmax is invariant to per-query constants,
    and q@bk is constant per query row.
  - attention, per (head-pair, q-block of 512): scores^T via two
    row-packed K=64 matmuls per k-tile (two heads run concurrently on
    the 128-row PE array); exp(x/8) on the scalar engine straight out of
    a 2-bank PSUM group; attn@V accumulates [65, 512] per head in PSUM.
    The k-tile loop is software-pipelined: scores(kt+1) is emitted ahead
    of attn@V(kt) so the PE never heads the queue with an exp-wait.
  - normalization: one fast DVE copy releases the accumulator, then
    reciprocal_approx_fast + gpsimd partition-broadcast + multiply.
  - output projection is cut into 8 units per q-block and interleaved
    into the next q-block's groups so its matmuls, PSUM->SBUF copies and
    output DMA all hide under the exp pipeline.
  - weights are host-prepacked into SBUF layout so every const DMA is a
    contiguous burst; DMA issue is spread across the sync/vector/gpsimd
    queues so the scalar queue stays clear for the exp pipeline.
The scalar engine paces the steady state (one 1.11us exp instruction per
k-tile); the PE array paces q-block 0 (projections are fused into its
groups) and is near-saturated elsewhere.
"""

import os
import numpy as np
import ml_dtypes

import concourse.bass as bass
import concourse.bacc as bacc
import concourse.mybir as mybir
import concourse.tile as tile
from concourse.bass_utils import run_bass_kernel_spmd

F32 = mybir.dt.float32
F32R = mybir.dt.float32r
BF16 = mybir.dt.bfloat16
AF = mybir.ActivationFunctionType

B, S, D = 2, 2048, 1024
H, DK = 16, 64
G = 4                  # head-groups (tensor parallel across cores)
DG = D // G            # 256 features per core
HPG = H // G           # 4 heads per core (2 row-packed pairs)
VEXT = HPG * (DK + 1)  # 260: per head [64 vp dims | 1 ones column]
P = 128
N_CORES = 8

_NC = None


def _build_program():
    nc = bacc.Bacc("TRN2", target_bir_lowering=False)
    qT = nc.dram_tensor("qT", [D, S], BF16, kind="ExternalInput")
    kT = nc.dram_tensor("kT", [D, S], BF16, kind="ExternalInput")
    vT = nc.dram_tensor("vT", [D, S], BF16, kind="ExternalInput")
    # host-prepacked SBUF layouts: contiguous per-partition bursts
    wqP = nc.dram_tensor("wqP", [P, 8 * DG], BF16, kind="ExternalInput")
    wkP = nc.dram_tensor("wkP", [P, 8 * DG], BF16, kind="ExternalInput")
    wvP = nc.dram_tensor("wvP", [P, 8 * VEXT], BF16, kind="ExternalInput")
    wvb = nc.dram_tensor("wvb", [1, VEXT], BF16, kind="ExternalInput")
    onesr = nc.dram_tensor("onesr", [1, P], BF16, kind="ExternalInput")
    woP = nc.dram_tensor("woP", [P, 2 * D], BF16, kind="ExternalInput")
    bqP = nc.dram_tensor("bqP", [P, 2], F32, kind="ExternalInput")
    out = nc.dram_tensor("out", [S, D], F32, kind="ExternalOutput")

    with tile.TileContext(nc) as tc:
        _body(nc, tc, qT, kT, vT, wqP, wkP, wvP, wvb, onesr, woP, bqP, out)
    nc.compile()
    return nc


def _body(nc, tc, qT, kT, vT, wqP, wkP, wvP, wvb, onesr, woP, bqP, out):
    with (
        tc.tile_pool(name="consts", bufs=1) as consts,
        tc.tile_pool(name="persist", bufs=1) as persist,
        tc.tile_pool(name="stage", bufs=6) as stage,
        tc.tile_pool(name="etp", bufs=8) as etp,
        tc.tile_pool(name="small", bufs=4) as small,
        tc.tile_pool(name="outp", bufs=8) as outp,
        tc.tile_pool(name="psA", bufs=2, space="PSUM") as psA,
        tc.tile_pool(name="psG", bufs=2, space="PSUM") as psG,
        tc.tile_pool(name="psC", bufs=1, space="PSUM") as psC,
    ):
        # --- constants / weights ---
        # critical path first, spread across queues: the first matmul needs
        # wk + the kT j0 x-block; qp needs wq + qT j0; vp needs wv + vtb.
        wk_sb = consts.tile([P, 8, DG], BF16)
        nc.scalar.dma_start(wk_sb[:], wkP[:].rearrange("p (t m) -> p t m", t=8))
        wq_sb = consts.tile([P, 8, DG], BF16)
        nc.scalar.dma_start(wq_sb[:], wqP[:].rearrange("p (t m) -> p t m", t=8))
        wv_sb = consts.tile([P, 8, VEXT], BF16)
        nc.scalar.dma_start(wv_sb[:], wvP[:].rearrange("p (t m) -> p t m", t=8))
        bq_sb = consts.tile([P, 2], F32)
        nc.scalar.dma_start(bq_sb[:], bqP[:])
        wvb_sb = consts.tile([1, VEXT], BF16)
        nc.scalar.dma_start(wvb_sb[:], wvb[:])
        ones_sb = consts.tile([1, P], BF16)
        nc.scalar.dma_start(ones_sb[:], onesr[:])

        # --- persistent activations ---
        qpT_sb = persist.tile([P, 2, S], BF16)   # [d%128, d-tile(=pair), s]
        kpT_sb = persist.tile([P, 2, S], BF16)
        vp_sb = persist.tile([P, 16, VEXT], BF16)  # [s%128, s-tile, 4*(64+1)]
        an_sb = persist.tile([P, 2, S], BF16)   # normalized attn output^T

        GRP = 2  # PSUM banks per exp group (one kt, both heads)
        def ps_alloc(n, i=[0]):
            i[0] += 1
            if i[0] % 2:
                return psA.tile([P, 512], F32, tag="a", name="ps_mm")[:, :n]
            return psG.tile([P, GRP * 512], F32, tag="g", name="gps")[:, :n]

        proj_xb = {}

        def proj_dma(src_t, j, eng=None):
            xb = stage.tile([P, 8, 512], BF16, tag="xb", name="xb")
            (eng or nc.sync).dma_start(
                xb[:],
                src_t[:].rearrange("(t p) s -> p t s", p=P)[
                    :, :, j * 512 : (j + 1) * 512
                ],
            )
            return xb

        def proj_half(src_t, w_sb, b_sb, dst, j, dt):
            key = (id(src_t), j)
            if key not in proj_xb:
                proj_xb[key] = proj_dma(src_t, j)
            xb = proj_xb[key]
            ps = ps_alloc(512)
            for kt in range(8):
                nc.tensor.matmul(
                    ps[:],
                    lhsT=w_sb[:, kt, dt * P : (dt + 1) * P],
                    rhs=xb[:, kt, :],
                    start=(kt == 0),
                    stop=(kt == 7),
                )
            if b_sb is None:
                nc.vector.tensor_copy(dst[:, dt, j * 512 : (j + 1) * 512], ps[:])
            else:
                nc.vector.tensor_scalar_add(
                    dst[:, dt, j * 512 : (j + 1) * 512], ps[:], b_sb[:, dt : dt + 1]
                )

        vtb_cache = {}

        def vp_block(st):
            # two s-tiles per DMA: 1KB bursts instead of 512B, half the loads
            st0 = st - st % 2
            if st0 not in vtb_cache:
                vtb2 = stage.tile([P, 8, 2 * P], BF16, tag="vtb", name="vtb")
                nc.sync.dma_start(
                    vtb2[:],
                    vT[:].rearrange("(t p) s -> p t s", p=P)[
                        :, :, st0 * P : (st0 + 2) * P
                    ],
                )
                vtb_cache[st0] = vtb2
            vtb = vtb_cache[st0]
            off = (st - st0) * P
            psv = ps_alloc(VEXT)
            for kt in range(8):
                nc.tensor.matmul(
                    psv[:],
                    lhsT=vtb[:, kt, off : off + P],
                    rhs=wv_sb[:, kt, :],
                    start=(kt == 0),
                    stop=(kt == 7),
                )
            nc.vector.tensor_tensor(
                vp_sb[:, st, :], psv[:], wvb_bc[:], mybir.AluOpType.add
            )

        wo_sb = consts.tile([P, 2, D], BF16)
        nc.scalar.dma_start(wo_sb[:], woP[:].rearrange("p (t o) -> p t o", t=2))

        # prefetch the kT/qT j0 x-blocks in interleaved halves so the qp
        # projection inputs land ~3us earlier than a serial 2MB load
        xb_k = stage.tile([P, 8, 512], BF16, tag="xb", name="xb")
        xb_q = stage.tile([P, 8, 512], BF16, tag="xb", name="xb")
        for half in range(2):
            hs = slice(4 * half, 4 * half + 4)
            for src_t, xb in ((kT, xb_k), (qT, xb_q)):
                nc.sync.dma_start(
                    xb[:, hs, :],
                    src_t[:].rearrange("(t p) s -> p t s", p=P)[:, hs, 0:512],
                )
        proj_xb[(id(kT), 0)] = xb_k
        proj_xb[(id(qT), 0)] = xb_q

        # bias+ones row broadcast once to all partitions (folded into the
        # vp PSUM->SBUF copy as a vector add, replacing 16 K=1 matmuls)
        wvb_bc = consts.tile([P, VEXT], BF16)
        nc.gpsimd.partition_broadcast(wvb_bc[:], wvb_sb[:])

        # warm the ACT exp table early so the ~2.7us load overlaps phase 1
        warm = consts.tile([1, 8], F32)
        nc.vector.memset(warm[:], 0.0)
        nc.scalar.activation(warm[:], warm[:], AF.Exp)



        # bootstrap: kp/qp j0 with the two accumulation chains interleaved
        # in DMA-half order, so qp starts as soon as its first half lands
        pk = ps_alloc(512)
        pq = ps_alloc(512)
        for half in range(2):
            for ps, xb, w_sb in ((pk, xb_k, wk_sb), (pq, xb_q, wq_sb)):
                for kt in range(4 * half, 4 * half + 4):
                    nc.tensor.matmul(
                        ps[:],
                        lhsT=w_sb[:, kt, 0:P],
                        rhs=xb[:, kt, :],
                        start=(kt == 0),
                        stop=(kt == 7),
                    )
        nc.vector.tensor_copy(kpT_sb[:, 0, 0:512], pk[:])
        nc.vector.tensor_scalar_add(qpT_sb[:, 0, 0:512], pq[:], bq_sb[:, 0:1])

        # insertion schedule for qb0/pair0: at group kt, emit these blocks
        # NOTE on schedule legality: scores(step+2) is emitted at step's
        # top (depth-2 hoist), so any kp/qp block consumed by the scores
        # of group g must be emitted at step g-3 or earlier.
        fuse0 = {kt: [] for kt in range(16)}
        for kt in range(14):
            fuse0[kt].append(("vp", kt + 2))
        for j in (1, 2, 3):
            fuse0[4 * j - 3].append(("kp", j, 0))   # needed at group 4j
        fuse0[2].append(("kp", 0, 1))               # pair1 kp j0
        fuse0[11].append(("qp", 0, 1))              # qp j0 dt1 for pair1
        # pair1's kp j1-3 stream just-in-time inside qb0/pair1 instead
        fuse1 = {kt: [] for kt in range(16)}
        for j in (1, 2, 3):
            fuse1[4 * j - 3].append(("kp", j, 1))

        # --- phase 2 per q-block; qp^T j-block emitted just-in-time ---
        def d_unit(qb, u, ring=0, final=False):
            # one (q-tile, out-half) unit of the output projection for block qb
            qt, o = u // 2, u % 2
            q0 = qb * 512 + qt * P
            dps = psA.tile([P, 512], F32, tag="a", name="dps")
            for p2 in range(2):
                nc.tensor.matmul(
                    dps[:],
                    lhsT=an_sb[:, p2, q0 : q0 + P],
                    rhs=wo_sb[:, p2, o * 512 : (o + 1) * 512],
                    start=(p2 == 0),
                    stop=(p2 == 1),
                )
            osb = outp.tile([P, 512], F32, tag="o")
            nc.vector.tensor_copy(osb[:], dps[:])
            eng = (nc.sync, nc.scalar if final else nc.gpsimd)[ring % 2]
            eng.dma_start(out[q0 : q0 + P, o * 512 : (o + 1) * 512], osb[:])

        # flat software-pipelined schedule over all (qb, pair, kt) steps:
        # scores(step+1) is emitted ahead of exp/attnV(step).
        steps = [(qb, pair, kt) for qb in range(4) for pair in range(2)
                 for kt in range(16)]
        cc_tiles = {}

        def get_cc(qb, pair):
            key = (qb, pair)
            if key not in cc_tiles:
                cc_tiles[key] = psC.tile([DK + 1, 1024], F32, tag="c", name="cc")
            return cc_tiles[key]

        def emit_scores(qb, pair, kt):
            qs = slice(qb * 512, (qb + 1) * 512)
            gps = psG.tile([P, GRP * 512], F32, tag="g", name="gps")
            for hh in range(2):
                hp = slice(hh * DK, (hh + 1) * DK)
                nc.tensor.matmul(
                    gps[:, hh * 512 : (hh + 1) * 512],
                    lhsT=kpT_sb[hp, pair, kt * P : (kt + 1) * P],
                    rhs=qpT_sb[hp, pair, qs],
                    start=True,
                    stop=True,
                )
            return gps

        def c_mms(qb, pair, kt, et):
            cc = get_cc(qb, pair)
            for hh in range(2):
                h = 2 * pair + hh
                nc.tensor.matmul(
                    cc[:, hh * 512 : (hh + 1) * 512],
                    lhsT=vp_sb[:, kt, h * (DK + 1) : (h + 1) * (DK + 1)],
                    rhs=et[:, hh * 512 : (hh + 1) * 512],
                    start=(kt == 0),
                    stop=(kt == 15),
                )

        def normalize(qb, pair):
            # single fast copy releases the PSUM accumulator; normalize
            # (reciprocal of row 64, broadcast, multiply) runs from SBUF
            qs = slice(qb * 512, (qb + 1) * 512)
            cc = cc_tiles.pop((qb, pair))
            csb = small.tile([DK + 1, 1024], F32, tag="csb")
            nc.vector.tensor_copy(csb[:], cc[:])
            rbcs = []
            for hh in range(2):
                cs = slice(hh * 512, (hh + 1) * 512)
                rsum = small.tile([1, 512], F32, tag="rsum")
                nc.vector.tensor_copy(rsum[:], csb[DK : DK + 1, cs])
                rinv = small.tile([1, 512], F32, tag="rinv")
                nc.vector.reciprocal_approx_fast(rinv[:], rsum[:])
                rbc = small.tile([DK, 512], F32, tag="rbc")
                nc.gpsimd.partition_broadcast(rbc[:], rinv[:])
                rbcs.append(rbc)
            for hh in range(2):
                cs = slice(hh * 512, (hh + 1) * 512)
                nc.vector.tensor_tensor(
                    an_sb[hh * DK : (hh + 1) * DK, pair, qs],
                    csb[:DK, cs],
                    rbcs[hh],
                    mybir.AluOpType.mult,
                )

        def normalize_final(qb, pair):
            # last pair: no successor waits on the PSUM banks, so read PSUM
            # directly; the reciprocal broadcast runs on the (now idle) PE
            # as a K=1 f32r matmul, a deferred qb2 output unit fills the
            # reciprocal-chain gap, and the multiply is cut into q-halves
            # so the output projection starts ASAP and the PE stays warm.
            qs0 = qb * 512
            cc = cc_tiles.pop((qb, pair))
            rinvs = []
            for hh in range(2):
                rsum = small.tile([1, 512], F32, tag="rsum")
                nc.vector.tensor_copy(rsum[:], cc[DK : DK + 1, hh * 512 : (hh + 1) * 512])
                rinv = small.tile([1, 512], F32, tag="rinv")
                nc.vector.reciprocal_approx_fast(rinv[:], rsum[:])
                rinvs.append(rinv)
            d_unit(qb - 1, 7, ring=0, final=True)
            rbcs = []
            for hh in range(2):
                rbc = small.tile([DK, 512], F32, tag="rbc")
                nc.gpsimd.partition_broadcast(rbc[:], rinvs[hh][:])
                rbcs.append(rbc)
            for half in range(2):
                h0 = half * 256
                for hh in range(2):
                    nc.vector.tensor_tensor(
                        an_sb[hh * DK : (hh + 1) * DK, pair,
                              qs0 + h0 : qs0 + h0 + 256],
                        cc[:DK, hh * 512 + h0 : hh * 512 + h0 + 256],
                        rbcs[hh][:, h0 : h0 + 256],
                        mybir.AluOpType.mult,
                    )
                for qt in (2 * half, 2 * half + 1):
                    d_unit(qb, 2 * qt, ring=qt, final=True)
                    d_unit(qb, 2 * qt + 1, ring=qt + 1, final=True)

        # spread the previous q-block's output projection through each
        # pair's groups so the scalar engine never starves; the last unit
        # of each stream for qb2 is deferred into qb3's endgame gaps
        d_sched = {3: 0, 7: 1, 11: 2, 14: 3}
        d_sched_qb3 = {3: 0, 7: 1, 11: 2}

        # depth-2 scores hoist: the PE queue always holds the next two
        # score groups ahead of the exp-gated attn@V matmuls
        gps_q = [emit_scores(*steps[0]), emit_scores(*steps[1])]
        for si, (qb, pair, kt) in enumerate(steps):
            if si == 0:
                vp_block(0)
                vp_block(1)
            gps = gps_q.pop(0)
            if si + 2 < len(steps):
                gps_q.append(emit_scores(*steps[si + 2]))
            et = etp.tile([P, GRP * 512], BF16, tag="e", name="et")
            nc.scalar.activation(et[:], gps[:], AF.Exp, scale=1.0 / np.sqrt(DK))
            c_mms(qb, pair, kt, et)
            sched = d_sched_qb3 if qb == 3 else d_sched
            if qb > 0 and kt in sched:
                u = 4 * pair + sched[kt]
                d_unit(qb - 1, u, ring=u)
            if qb == 0 and pair == 0:
                for item in fuse0[kt]:
                    if item[0] == "vp":
                        vp_block(item[1])
                    elif item[0] == "kp":
                        proj_half(kT, wk_sb, None, kpT_sb, item[1], item[2])
                    else:
                        proj_half(qT, wq_sb, bq_sb, qpT_sb, item[1], item[2])
            if qb == 0 and pair == 1:
                for item in fuse1[kt]:
                    proj_half(kT, wk_sb, None, kpT_sb, item[1], item[2])
            if pair == 1 and qb < 3 and kt in (1, 9):
                proj_half(qT, wq_sb, bq_sb, qpT_sb, qb + 1, kt // 8)
            if kt == 15:
                if (qb, pair) == (3, 1):
                    normalize_final(qb, pair)
                elif (qb, pair) == (3, 0):
                    normalize(qb, pair)
                    d_unit(2, 3, ring=3)
                else:
                    normalize(qb, pair)


def _get_program():
    global _NC
    if _NC is None:
        _NC = _build_program()
    return _NC


def _make_in_maps(v, k, q, Wv, bv, Wk, bk, Wq, bq, Wo, bo):
    f32 = np.float32
    bf16 = ml_dtypes.bfloat16
    qT = [np.ascontiguousarray(q[b].T).astype(bf16) for b in range(B)]
    kT = [np.ascontiguousarray(k[b].T).astype(bf16) for b in range(B)]
    vT = [np.ascontiguousarray(v[b].T).astype(bf16) for b in range(B)]

    def prepack(w2d):
        # [D, M] -> [P, 8*M]: row t*128+p lands at partition p, block t
        Dd, M = w2d.shape
        return np.ascontiguousarray(
            w2d.reshape(8, P, M).transpose(1, 0, 2).reshape(P, 8 * M)
        )

    per_group = []
    for g in range(G):
        gs = slice(g * DG, (g + 1) * DG)
        wqT = prepack(Wq[gs, :].T.astype(f32)).astype(bf16)
        wkT = prepack(Wk[gs, :].T.astype(f32)).astype(bf16)
        wvm = np.zeros((D, VEXT), dtype=f32)
        wvb = np.zeros((1, VEXT), dtype=f32)
        for h in range(HPG):
            cs = slice(h * (DK + 1), h * (DK + 1) + DK)
            rows = slice(g * DG + h * DK, g * DG + (h + 1) * DK)
            wvm[:, cs] = Wv[rows, :].T
            wvb[0, cs] = bv[rows]
            wvb[0, h * (DK + 1) + DK] = 1.0
        wvP = prepack(wvm).astype(bf16)
        wvb = wvb.astype(bf16)
        # [DG, D] -> [P, 2*D]
        woT = np.ascontiguousarray(Wo[:, gs].T)
        woP = np.ascontiguousarray(
            woT.reshape(2, P, D).transpose(1, 0, 2).reshape(P, 2 * D)
        ).astype(bf16)
        bqP = np.ascontiguousarray(
            bq[gs].reshape(2, P).T, dtype=f32
        )
        per_group.append(
            dict(wqP=wqT, wkP=wkT, wvP=wvP, wvb=wvb, woP=woP, bqP=bqP)
        )

    in_maps = []
    for c in range(N_CORES):
        b, g = c // G, c % G
        m = dict(qT=qT[b], kT=kT[b], vT=vT[b],
                 onesr=np.ones((1, P), dtype=bf16), **per_group[g])
        in_maps.append(m)
    return in_maps


def _gather(results, bo):
    out = np.zeros((B, S, D), dtype=np.float32)
    for c in range(N_CORES):
        b = c // G
        out[b] += results[c]["out"]
    out += bo.astype(np.float32)
    return out


def run(v, k, q, Wv, bv, Wk, bk, Wq, bq, Wo, bo, trace=False):
    nc = _get_program()
    in_maps = _make_in_maps(v, k, q, Wv, bv, Wk, bk, Wq, bq, Wo, bo)
    res = run_bass_kernel_spmd(
        nc, in_maps, core_ids=list(range(N_CORES)), trace=trace
    )
    return _gather(res.results, np.asarray(bo)), res


def kernel(v, k, q, Wv, bv, Wk, bk, Wq, bq, Wo, bo):
    args = [np.asarray(x, dtype=np.float32)
            for x in (v, k, q, Wv, bv, Wk, bk, Wq, bq, Wo, bo)]
    out, _ = run(*args, trace=bool(int(os.environ.get("MHA_TRACE", "0"))))
    return out


# revision 30
# speedup vs baseline: 1.0707x; 1.0070x over previous
"""Trainium2 Bass kernel for multi-head attention (B=2, S=2048, D=1024, H=16).

Sharding: 8 cores = 2 (batch, data-parallel) x 4 (head-groups, tensor-parallel).
Each core (b, g) handles batch b and heads [4g, 4g+4) (a 256-wide slice of the
model dim), computing a partial output contribution; the host sums the 4
head-group partials per batch and adds the output bias.

Per-core pipeline (everything bf16 into fp32 PSUM accumulation):
  - projections: qp^T/kp^T feature-major ([d, s], W^T stationary) so the
    attention matmuls need no transposes; vp sequence-major with a fused
    bias row and a ones column per head (the ones column makes attn@V
    emit the softmax row-sums for free as PSUM row 64). The K-projection
    bias is dropped entirely: softmax is invariant to per-query constants,
    and q@bk is constant per query row.
  - attention, per (head-pair, q-block of 512): scores^T via two
    row-packed K=64 matmuls per k-tile (two heads run concurrently on
    the 128-row PE array); exp(x/8) on the scalar engine straight out of
    a 2-bank PSUM group; attn@V accumulates [65, 512] per head in PSUM.
    The k-tile loop is software-pipelined: scores(kt+1) is emitted ahead
    of attn@V(kt) so the PE never heads the queue with an exp-wait.
  - normalization: one fast DVE copy releases the accumulator, then
    reciprocal_approx_fast + gpsimd partition-broadcast + multiply.
  - output projection is cut into 8 units per q-block and interleaved
    into the next q-block's groups so its matmuls, PSUM->SBUF copies and
    output DMA all hide under the exp pipeline.
  - weights are host-prepacked into SBUF layout so every const DMA is a
    contiguous burst; DMA issue is spread across the sync/vector/gpsimd
    queues so the scalar queue stays clear for the exp pipeline.
The scalar engine paces the steady state (one 1.11us exp instruction per
k-tile); the PE array paces q-block 0 (projections are fused into its
groups) and is near-saturated elsewhere.
"""

import os
import numpy as np
import ml_dtypes

import concourse.bass as bass
import concourse.bacc as bacc
import concourse.mybir as mybir
import concourse.tile as tile
from concourse.bass_utils import run_bass_kernel_spmd

F32 = mybir.dt.float32
F32R = mybir.dt.float32r
BF16 = mybir.dt.bfloat16
AF = mybir.ActivationFunctionType

B, S, D = 2, 2048, 1024
H, DK = 16, 64
G = 4                  # head-groups (tensor parallel across cores)
DG = D // G            # 256 features per core
HPG = H // G           # 4 heads per core (2 row-packed pairs)
VEXT = HPG * (DK + 1)  # 260: per head [64 vp dims | 1 ones column]
P = 128
N_CORES = 8

_NC = None


def _build_program():
    nc = bacc.Bacc("TRN2", target_bir_lowering=False)
    qT = nc.dram_tensor("qT", [D, S], BF16, kind="ExternalInput")
    kT = nc.dram_tensor("kT", [D, S], BF16, kind="ExternalInput")
    vT = nc.dram_tensor("vT", [D, S], BF16, kind="ExternalInput")
    # host-prepacked SBUF layouts: contiguous per-partition bursts
    wqP = nc.dram_tensor("wqP", [P, 8 * DG], BF16, kind="ExternalInput")
    wkP = nc.dram_tensor("wkP", [P, 8 * DG], BF16, kind="ExternalInput")
    wvP = nc.dram_tensor("wvP", [P, 8 * VEXT], BF16, kind="ExternalInput")
    wvb = nc.dram_tensor("wvb", [1, VEXT], BF16, kind="ExternalInput")
    onesr = nc.dram_tensor("onesr", [1, P], BF16, kind="ExternalInput")
    woP = nc.dram_tensor("woP", [P, 2 * D], BF16, kind="ExternalInput")
    bqP = nc.dram_tensor("bqP", [P, 2], F32, kind="ExternalInput")
    out = nc.dram_tensor("out", [S, D], F32, kind="ExternalOutput")

    with tile.TileContext(nc) as tc:
        _body(nc, tc, qT, kT, vT, wqP, wkP, wvP, wvb, onesr, woP, bqP, out)
    nc.compile()
    return nc


def _body(nc, tc, qT, kT, vT, wqP, wkP, wvP, wvb, onesr, woP, bqP, out):
    with (
        tc.tile_pool(name="consts", bufs=1) as consts,
        tc.tile_pool(name="persist", bufs=1) as persist,
        tc.tile_pool(name="stage", bufs=6) as stage,
        tc.tile_pool(name="etp", bufs=8) as etp,
        tc.tile_pool(name="small", bufs=4) as small,
        tc.tile_pool(name="outp", bufs=8) as outp,
        tc.tile_pool(name="psA", bufs=2, space="PSUM") as psA,
        tc.tile_pool(name="psG", bufs=2, space="PSUM") as psG,
        tc.tile_pool(name="psC", bufs=1, space="PSUM") as psC,
    ):
        # --- constants / weights ---
        # critical path first, spread across queues: the first matmul needs
        # wk + the kT j0 x-block; qp needs wq + qT j0; vp needs wv + vtb.
        wk_sb = consts.tile([P, 8, DG], BF16)
        nc.scalar.dma_start(wk_sb[:], wkP[:].rearrange("p (t m) -> p t m", t=8))
        wq_sb = consts.tile([P, 8, DG], BF16)
        nc.scalar.dma_start(wq_sb[:], wqP[:].rearrange("p (t m) -> p t m", t=8))
        wv_sb = consts.tile([P, 8, VEXT], BF16)
        nc.scalar.dma_start(wv_sb[:], wvP[:].rearrange("p (t m) -> p t m", t=8))
        bq_sb = consts.tile([P, 2], F32)
        nc.scalar.dma_start(bq_sb[:], bqP[:])
        wvb_sb = consts.tile([1, VEXT], BF16)
        nc.scalar.dma_start(wvb_sb[:], wvb[:])
        ones_sb = consts.tile([1, P], BF16)
        nc.scalar.dma_start(ones_sb[:], onesr[:])

        # --- persistent activations ---
        qpT_sb = persist.tile([P, 2, S], BF16)   # [d%128, d-tile(=pair), s]
        kpT_sb = persist.tile([P, 2, S], BF16)
        vp_sb = persist.tile([P, 16, VEXT], BF16)  # [s%128, s-tile, 4*(64+1)]
        an_sb = persist.tile([P, 2, S], BF16)   # normalized attn output^T

        GRP = 2  # PSUM banks per exp group (one kt, both heads)
        def ps_alloc(n, i=[0]):
            i[0] += 1
            if i[0] % 2:
                return psA.tile([P, 512], F32, tag="a", name="ps_mm")[:, :n]
            return psG.tile([P, GRP * 512], F32, tag="g", name="gps")[:, :n]

        proj_xb = {}

        def proj_dma(src_t, j, eng=None):
            xb = stage.tile([P, 8, 512], BF16, tag="xb", name="xb")
            (eng or nc.sync).dma_start(
                xb[:],
                src_t[:].rearrange("(t p) s -> p t s", p=P)[
                    :, :, j * 512 : (j + 1) * 512
                ],
            )
            return xb

        def proj_half(src_t, w_sb, b_sb, dst, j, dt):
            key = (id(src_t), j)
            if key not in proj_xb:
                proj_xb[key] = proj_dma(src_t, j)
            xb = proj_xb[key]
            ps = ps_alloc(512)
            for kt in range(8):
                nc.tensor.matmul(
                    ps[:],
                    lhsT=w_sb[:, kt, dt * P : (dt + 1) * P],
                    rhs=xb[:, kt, :],
                    start=(kt == 0),
                    stop=(kt == 7),
                )
            if b_sb is None:
                nc.vector.tensor_copy(dst[:, dt, j * 512 : (j + 1) * 512], ps[:])
            else:
                nc.vector.tensor_scalar_add(
                    dst[:, dt, j * 512 : (j + 1) * 512], ps[:], b_sb[:, dt : dt + 1]
                )

        vtb_cache = {}

        def vp_block(st):
            # two s-tiles per DMA: 1KB bursts instead of 512B, half the loads
            st0 = st - st % 2
            if st0 not in vtb_cache:
                vtb2 = stage.tile([P, 8, 2 * P], BF16, tag="vtb", name="vtb")
                nc.sync.dma_start(
                    vtb2[:],
                    vT[:].rearrange("(t p) s -> p t s", p=P)[
                        :, :, st0 * P : (st0 + 2) * P
                    ],
                )
                vtb_cache[st0] = vtb2
            vtb = vtb_cache[st0]
            off = (st - st0) * P
            psv = ps_alloc(VEXT)
            for kt in range(8):
                nc.tensor.matmul(
                    psv[:],
                    lhsT=vtb[:, kt, off : off + P],
                    rhs=wv_sb[:, kt, :],
                    start=(kt == 0),
                    stop=(kt == 7),
                )
            nc.vector.tensor_tensor(
                vp_sb[:, st, :], psv[:], wvb_bc[:], mybir.AluOpType.add
            )

        wo_sb = consts.tile([P, 2, D], BF16)
        nc.scalar.dma_start(wo_sb[:], woP[:].rearrange("p (t o) -> p t o", t=2))

        # prefetch the kT/qT j0 x-blocks on the sync queue right away
        xb_k = proj_dma(kT, 0, eng=nc.sync)
        xb_q = proj_dma(qT, 0, eng=nc.sync)
        proj_xb[(id(kT), 0)] = xb_k
        proj_xb[(id(qT), 0)] = xb_q

        # warm the PE HAM clock gate with throwaway matmuls while the
        # first weight/activation DMAs are still in flight, so the
        # bootstrap projections run at 2.4GHz instead of 1.2
        junk = stage.tile([P, 640], BF16, tag="junk")
        nc.vector.memset(junk[:], 0.0)
        jps = psA.tile([P, 512], F32, tag="a", name="ps_mm")
        for _ in range(10):
            nc.tensor.matmul(
                jps[:], lhsT=junk[:, 0:P], rhs=junk[:, P:640],
                start=True, stop=True,
            )

        # bias+ones row broadcast once to all partitions (folded into the
        # vp PSUM->SBUF copy as a vector add, replacing 16 K=1 matmuls)
        wvb_bc = consts.tile([P, VEXT], BF16)
        nc.gpsimd.partition_broadcast(wvb_bc[:], wvb_sb[:])

        # warm the ACT exp table early so the ~2.7us load overlaps phase 1
        warm = consts.tile([1, 8], F32)
        nc.vector.memset(warm[:], 0.0)
        nc.scalar.activation(warm[:], warm[:], AF.Exp)



        # bootstrap: kp/qp j0 with the two accumulation chains interleaved
        # in DMA-half order, so qp starts as soon as its first half lands
        pk = ps_alloc(512)
        pq = ps_alloc(512)
        for half in range(2):
            for ps, xb, w_sb in ((pk, xb_k, wk_sb), (pq, xb_q, wq_sb)):
                for kt in range(4 * half, 4 * half + 4):
                    nc.tensor.matmul(
                        ps[:],
                        lhsT=w_sb[:, kt, 0:P],
                        rhs=xb[:, kt, :],
                        start=(kt == 0),
                        stop=(kt == 7),
                    )
        nc.vector.tensor_copy(kpT_sb[:, 0, 0:512], pk[:])
        nc.vector.tensor_scalar_add(qpT_sb[:, 0, 0:512], pq[:], bq_sb[:, 0:1])

        # insertion schedule for qb0/pair0: at group kt, emit these blocks
        # NOTE on schedule legality: scores(step+2) is emitted at step's
        # top (depth-2 hoist), so any kp/qp block consumed by the scores
        # of group g must be emitted at step g-3 or earlier.
        fuse0 = {kt: [] for kt in range(16)}
        for kt in range(14):
            fuse0[kt].append(("vp", kt + 2))
        for j in (1, 2, 3):
            fuse0[4 * j - 3].append(("kp", j, 0))   # needed at group 4j
        fuse0[2].append(("kp", 0, 1))               # pair1 kp j0
        fuse0[11].append(("qp", 0, 1))              # qp j0 dt1 for pair1
        # pair1's kp j1-3 stream just-in-time inside qb0/pair1 instead
        fuse1 = {kt: [] for kt in range(16)}
        for j in (1, 2, 3):
            fuse1[4 * j - 3].append(("kp", j, 1))

        # --- phase 2 per q-block; qp^T j-block emitted just-in-time ---
        def d_unit(qb, u, ring=0, final=False):
            # one (q-tile, out-half) unit of the output projection for block qb
            qt, o = u // 2, u % 2
            q0 = qb * 512 + qt * P
            dps = psA.tile([P, 512], F32, tag="a", name="dps")
            for p2 in range(2):
                nc.tensor.matmul(
                    dps[:],
                    lhsT=an_sb[:, p2, q0 : q0 + P],
                    rhs=wo_sb[:, p2, o * 512 : (o + 1) * 512],
                    start=(p2 == 0),
                    stop=(p2 == 1),
                )
            osb = outp.tile([P, 512], F32, tag="o")
            nc.vector.tensor_copy(osb[:], dps[:])
            eng = (nc.sync, nc.scalar if final else nc.gpsimd)[ring % 2]
            eng.dma_start(out[q0 : q0 + P, o * 512 : (o + 1) * 512], osb[:])

        # flat software-pipelined schedule over all (qb, pair, kt) steps:
        # scores(step+1) is emitted ahead of exp/attnV(step).
        steps = [(qb, pair, kt) for qb in range(4) for pair in range(2)
                 for kt in range(16)]
        cc_tiles = {}

        def get_cc(qb, pair):
            key = (qb, pair)
            if key not in cc_tiles:
                cc_tiles[key] = psC.tile([DK + 1, 1024], F32, tag="c", name="cc")
            return cc_tiles[key]

        def emit_scores(qb, pair, kt):
            qs = slice(qb * 512, (qb + 1) * 512)
            gps = psG.tile([P, GRP * 512], F32, tag="g", name="gps")
            for hh in range(2):
                hp = slice(hh * DK, (hh + 1) * DK)
                nc.tensor.matmul(
                    gps[:, hh * 512 : (hh + 1) * 512],
                    lhsT=kpT_sb[hp, pair, kt * P : (kt + 1) * P],
                    rhs=qpT_sb[hp, pair, qs],
                    start=True,
                    stop=True,
                )
            return gps

        def c_mms(qb, pair, kt, et):
            cc = get_cc(qb, pair)
            for hh in range(2):
                h = 2 * pair + hh
                nc.tensor.matmul(
                    cc[:, hh * 512 : (hh + 1) * 512],
                    lhsT=vp_sb[:, kt, h * (DK + 1) : (h + 1) * (DK + 1)],
                    rhs=et[:, hh * 512 : (hh + 1) * 512],
                    start=(kt == 0),
                    stop=(kt == 15),
                )

        def normalize(qb, pair):
            # single fast copy releases the PSUM accumulator; normalize
            # (reciprocal of row 64, broadcast, multiply) runs from SBUF
            qs = slice(qb * 512, (qb + 1) * 512)
            cc = cc_tiles.pop((qb, pair))
            csb = small.tile([DK + 1, 1024], F32, tag="csb")
            nc.vector.tensor_copy(csb[:], cc[:])
            rbcs = []
            for hh in range(2):
                cs = slice(hh * 512, (hh + 1) * 512)
                rsum = small.tile([1, 512], F32, tag="rsum")
                nc.vector.tensor_copy(rsum[:], csb[DK : DK + 1, cs])
                rinv = small.tile([1, 512], F32, tag="rinv")
                nc.vector.reciprocal_approx_fast(rinv[:], rsum[:])
                rbc = small.tile([DK, 512], F32, tag="rbc")
                nc.gpsimd.partition_broadcast(rbc[:], rinv[:])
                rbcs.append(rbc)
            for hh in range(2):
                cs = slice(hh * 512, (hh + 1) * 512)
                nc.vector.tensor_tensor(
                    an_sb[hh * DK : (hh + 1) * DK, pair, qs],
                    csb[:DK, cs],
                    rbcs[hh],
                    mybir.AluOpType.mult,
                )

        def normalize_final(qb, pair):
            # last pair: no successor waits on the PSUM banks, so read PSUM
            # directly; the reciprocal broadcast runs on the (now idle) PE
            # as a K=1 f32r matmul, a deferred qb2 output unit fills the
            # reciprocal-chain gap, and the multiply is cut into q-halves
            # so the output projection starts ASAP and the PE stays warm.
            qs0 = qb * 512
            cc = cc_tiles.pop((qb, pair))
            # hh0's row-sum copy runs on the scalar engine (idle now) so
            # the two reciprocal chains overlap; two deferred qb2 output
            # units keep the PE busy and the HAM clock warm meanwhile
            rinvs = []
            for hh in range(2):
                rsum = small.tile([1, 512], F32, tag="rsum")
                if hh == 1:
                    nc.scalar.copy(rsum[:], cc[DK : DK + 1, hh * 512 : (hh + 1) * 512])
                else:
                    nc.vector.tensor_copy(rsum[:], cc[DK : DK + 1, hh * 512 : (hh + 1) * 512])
                rinv = small.tile([1, 512], F32, tag="rinv")
                nc.vector.reciprocal_approx_fast(rinv[:], rsum[:])
                rinvs.append(rinv)
            d_unit(qb - 1, 6, ring=0, final=True)
            rbcs = []
            for hh in range(2):
                rbc = small.tile([DK, 512], F32, tag="rbc")
                nc.gpsimd.partition_broadcast(rbc[:], rinvs[hh][:])
                rbcs.append(rbc)
            d_unit(qb - 1, 7, ring=1, final=True)
            for half in range(2):
                h0 = half * 256
                for hh in range(2):
                    nc.vector.tensor_tensor(
                        an_sb[hh * DK : (hh + 1) * DK, pair,
                              qs0 + h0 : qs0 + h0 + 256],
                        cc[:DK, hh * 512 + h0 : hh * 512 + h0 + 256],
                        rbcs[hh][:, h0 : h0 + 256],
                        mybir.AluOpType.mult,
                    )
                for qt in (2 * half, 2 * half + 1):
                    d_unit(qb, 2 * qt, ring=qt, final=True)
                    d_unit(qb, 2 * qt + 1, ring=qt + 1, final=True)

        # spread the previous q-block's output projection through each
        # pair's groups so the scalar engine never starves; the last unit
        # of each stream for qb2 is deferred into qb3's endgame gaps
        d_sched = {4: 0, 8: 1, 11: 2, 14: 3}
        d_sched_qb3_p0 = {4: 0, 8: 1, 11: 2}
        d_sched_qb3_p1 = {4: 0, 8: 1}

        # depth-2 scores hoist: the PE queue always holds the next two
        # score groups ahead of the exp-gated attn@V matmuls
        gps_q = [emit_scores(*steps[0]), emit_scores(*steps[1])]
        for si, (qb, pair, kt) in enumerate(steps):
            if si == 0:
                vp_block(0)
                vp_block(1)
            gps = gps_q.pop(0)
            if si + 2 < len(steps):
                gps_q.append(emit_scores(*steps[si + 2]))
            et = etp.tile([P, GRP * 512], BF16, tag="e", name="et")
            nc.scalar.activation(et[:], gps[:], AF.Exp, scale=1.0 / np.sqrt(DK))
            c_mms(qb, pair, kt, et)
            sched = ((d_sched_qb3_p1 if pair else d_sched_qb3_p0)
                     if qb == 3 else d_sched)
            if qb > 0 and kt in sched:
                u = 4 * pair + sched[kt]
                d_unit(qb - 1, u, ring=u)
            if qb == 0 and pair == 0:
                for item in fuse0[kt]:
                    if item[0] == "vp":
                        vp_block(item[1])
                    elif item[0] == "kp":
                        proj_half(kT, wk_sb, None, kpT_sb, item[1], item[2])
                    else:
                        proj_half(qT, wq_sb, bq_sb, qpT_sb, item[1], item[2])
            if qb == 0 and pair == 1:
                for item in fuse1[kt]:
                    proj_half(kT, wk_sb, None, kpT_sb, item[1], item[2])
            if pair == 1 and qb < 3 and kt in (1, 9):
                proj_half(qT, wq_sb, bq_sb, qpT_sb, qb + 1, kt // 8)
            if kt == 15:
                if (qb, pair) == (3, 1):
                    normalize_final(qb, pair)
                elif (qb, pair) == (3, 0):
                    normalize(qb, pair)
                    d_unit(2, 3, ring=3)
                else:
                    normalize(qb, pair)


def _get_program():
    global _NC
    if _NC is None:
        _NC = _build_program()
    return _NC


def _make_in_maps(v, k, q, Wv, bv, Wk, bk, Wq, bq, Wo, bo):
    f32 = np.float32
    bf16 = ml_dtypes.bfloat16
    qT = [np.ascontiguousarray(q[b].T).astype(bf16) for b in range(B)]
    kT = [np.ascontiguousarray(k[b].T).astype(bf16) for b in range(B)]
    vT = [np.ascontiguousarray(v[b].T).astype(bf16) for b in range(B)]

    def prepack(w2d):
        # [D, M] -> [P, 8*M]: row t*128+p lands at partition p, block t
        Dd, M = w2d.shape
        return np.ascontiguousarray(
            w2d.reshape(8, P, M).transpose(1, 0, 2).reshape(P, 8 * M)
        )

    per_group = []
    for g in range(G):
        gs = slice(g * DG, (g + 1) * DG)
        wqT = prepack(Wq[gs, :].T.astype(f32)).astype(bf16)
        wkT = prepack(Wk[gs, :].T.astype(f32)).astype(bf16)
        wvm = np.zeros((D, VEXT), dtype=f32)
        wvb = np.zeros((1, VEXT), dtype=f32)
        for h in range(HPG):
            cs = slice(h * (DK + 1), h * (DK + 1) + DK)
            rows = slice(g * DG + h * DK, g * DG + (h + 1) * DK)
            wvm[:, cs] = Wv[rows, :].T
            wvb[0, cs] = bv[rows]
            wvb[0, h * (DK + 1) + DK] = 1.0
        wvP = prepack(wvm).astype(bf16)
        wvb = wvb.astype(bf16)
        # [DG, D] -> [P, 2*D]
        woT = np.ascontiguousarray(Wo[:, gs].T)
        woP = np.ascontiguousarray(
            woT.reshape(2, P, D).transpose(1, 0, 2).reshape(P, 2 * D)
        ).astype(bf16)
        bqP = np.ascontiguousarray(
            bq[gs].reshape(2, P).T, dtype=f32
        )
        per_group.append(
            dict(wqP=wqT, wkP=wkT, wvP=wvP, wvb=wvb, woP=woP, bqP=bqP)
        )

    in_maps = []
    for c in range(N_CORES):
        b, g = c // G, c % G
        m = dict(qT=qT[b], kT=kT[b], vT=vT[b],
                 onesr=np.ones((1, P), dtype=bf16), **per_group[g])
        in_maps.append(m)
    return in_maps


def _gather(results, bo):
    out = np.zeros((B, S, D), dtype=np.float32)
    for c in range(N_CORES):
        b = c // G
        out[b] += results[c]["out"]
    out += bo.astype(np.float32)
    return out


def run(v, k, q, Wv, bv, Wk, bk, Wq, bq, Wo, bo, trace=False):
    nc = _get_program()
    in_maps = _make_in_maps(v, k, q, Wv, bv, Wk, bk, Wq, bq, Wo, bo)
    res = run_bass_kernel_spmd(
        nc, in_maps, core_ids=list(range(N_CORES)), trace=trace
    )
    return _gather(res.results, np.asarray(bo)), res


def kernel(v, k, q, Wv, bv, Wk, bk, Wq, bq, Wo, bo):
    args = [np.asarray(x, dtype=np.float32)
            for x in (v, k, q, Wv, bv, Wk, bk, Wq, bq, Wo, bo)]
    out, _ = run(*args, trace=bool(int(os.environ.get("MHA_TRACE", "0"))))
    return out
